# revision 7
# baseline (speedup 1.0000x reference)
"""AttentionMILPooling Trainium2 kernel.

Math (matches the jax reference):
    scores  = tanh(X @ W1 + b1) @ W2 + b2          # [T, 1]
    weights = softmax(scores, axis=0)              # global over all T
    out[b]  = sum_{i in bag b} weights[i] * X[i]   # [64, 512]

Key identities used:
  * b2 cancels exactly in the softmax (exp(s+b2)/sum exp(s+b2) = exp(s)/sum exp(s)).
  * scores are bounded (|s| <= sum|W2| ~ 13) so no max-subtraction is needed;
    exp stays well inside fp32 range.
  * out[b] = (sum_{i in b} exp(s_i) * X_i) / Z with Z = sum_i exp(s_i): each core
    computes unnormalized per-bag sums U and per-bag exp-sums z; the host sums z
    over all cores/bags and divides once.

Sharding: rows (instances) are split contiguously across 8 cores, 16384 rows
each; with the equal 2048-row bags every core owns exactly 8 whole bags.
The tiny MLP weights are replicated. Each core returns U^T [512 x 8] and
z [1 x 8]; the host concatenates, divides by the global Z.

Per-core device pipeline, per 256-row group (64 groups):
  DMA   : X rows fp32 -> SBUF bf16 (cast in SWDGE DMA)         [128,2,512]
  PE    : 8x transpose  X^T chunks -> PSUM (bf16)              [128,1024]
  DVE   : 2x copy PSUM->SBUF X^T                               [128,4,256]
  PE    : 8x matmul  H^T = W1^T @ X^T (accum over f chunks)    [128,512] f32
  ACT   : 4x tanh(H^T + b1) -> SBUF bf16
  PE    : 4x matmul  s = tanhH^T.T @ W2 (accum over h chunks)  [128,2] f32
  ACT   : 2x exp(s) -> w SBUF bf16
  PE    : 2x matmul  z[bag]  += ones.T @ w
  PE    : 8x matmul  U^T[:,bag] += X_chunk.T @ w  (per f chunk)
"""

import numpy as np

N_CORES = 8
T_FULL = 131072
F = 512  # feature dim
HID = 256  # hidden dim
B_FULL = 64  # number of bags
P = 128  # partitions

_COMPILED_CACHE = {}


def _build_program(n_tiles, tile_col, n_cols):
    """Build the SPMD bass program.

    n_tiles: number of 128-row tiles per core (must be even).
    tile_col: list, local bag-column index for each tile (same on all cores).
    n_cols: number of local bag columns.
    """
    import concourse.bass as bass
    import concourse.bacc as bacc
    import concourse.mybir as mybir
    from concourse.tile import TileContext
    from concourse.masks import make_identity

    f32 = mybir.dt.float32
    bf16 = mybir.dt.bfloat16
    FC = F // P  # 4 feature chunks
    MC = HID // P  # 2 hidden chunks
    rows_per_core = n_tiles * P
    n_groups = n_tiles // 2

    # first / last tile per column (for PSUM start/stop flags)
    first_tile = {}
    last_tile = {}
    for t, c in enumerate(tile_col):
        first_tile.setdefault(c, t)
        last_tile[c] = t

    nc = bacc.Bacc(
        "TRN2", target_bir_lowering=False, debug=False, num_devices=N_CORES
    )

    x = nc.declare_dram_parameter("x", [rows_per_core, F], f32, isOutput=False)
    w1 = nc.declare_dram_parameter("w1", [F, HID], f32, isOutput=False)
    b1 = nc.declare_dram_parameter("b1", [HID], f32, isOutput=False)
    w2 = nc.declare_dram_parameter("w2", [HID, 1], f32, isOutput=False)
    ut_out = nc.declare_dram_parameter("ut", [P, FC * n_cols], f32, isOutput=True)
    z_out = nc.declare_dram_parameter("z", [1, n_cols], f32, isOutput=True)

    with TileContext(nc) as tc:
        with (
            tc.tile_pool(name="const", bufs=1) as const_pool,
            tc.tile_pool(name="xb", bufs=3) as xb_pool,
            tc.tile_pool(name="xt", bufs=2) as xt_pool,
            tc.tile_pool(name="th", bufs=2) as th_pool,
            tc.tile_pool(name="wv", bufs=2) as wv_pool,
            tc.tile_pool(name="out_sb", bufs=1) as out_pool,
            tc.tile_pool(name="pt", bufs=2, space="PSUM") as pt_pool,
            tc.tile_pool(name="hp", bufs=2, space="PSUM") as hp_pool,
            tc.tile_pool(name="sp", bufs=2, space="PSUM") as sp_pool,
            tc.tile_pool(name="acc", bufs=1, space="PSUM") as acc_pool,
        ):
            # ---- constants ----
            ident = const_pool.tile([P, P], bf16)
            make_identity(nc, ident)
            ones = const_pool.tile([P, 1], bf16)
            nc.gpsimd.memset(ones, 1.0)
            zeros = const_pool.tile([P, FC], bf16)
            nc.gpsimd.memset(zeros, 0.0)

            # W1 -> bf16 chunks: w1b[p, c, m, j] = W1[c*128+p, m*128+j]
            w1f = const_pool.tile([P, FC, MC, P], f32)
            nc.sync.dma_start(
                out=w1f, in_=x_ap_rearr(w1, "(c p) (m j) -> p c m j", p=P, j=P)
            )
            w1b = const_pool.tile([P, FC, MC, P], bf16)
            nc.vector.tensor_copy(out=w1b, in_=w1f)

            # W2 -> bf16 chunks: w2b[p, m, 0] = W2[m*128+p, 0]
            w2f = const_pool.tile([P, MC, 1], f32)
            nc.sync.dma_start(
                out=w2f, in_=x_ap_rearr(w2, "(m p) one -> p m one", p=P)
            )
            w2b = const_pool.tile([P, MC, 1], bf16)
            nc.vector.tensor_copy(out=w2b, in_=w2f)

            # b1 per-partition bias: b1s[p, m] = b1[m*128+p]
            b1s = const_pool.tile([P, MC], f32)
            nc.sync.dma_start(out=b1s, in_=x_ap_rearr(b1, "(m p) -> p m", p=P))

            # ---- persistent accumulators ----
            # PSUM start=True zeroes a whole 2KB region, so each accumulator
            # holds ONE region-wide group open for the entire kernel: a dummy
            # zeroing matmul opens it, every bag-update accumulates with
            # start=False, and a dummy matmul closes it before readout.
            ut_psum = acc_pool.tile([P, FC * n_cols], f32)
            z_psum = acc_pool.tile([1, n_cols], f32)
            nc.tensor.matmul(
                ut_psum[:, 0 : FC * n_cols],
                ident,
                zeros[:, 0:1].broadcast_to([P, FC * n_cols]),
                start=True,
                stop=False,
            )
            nc.tensor.matmul(
                z_psum[0:1, 0:n_cols],
                ones,
                zeros[:, 0:1].broadcast_to([P, n_cols]),
                start=True,
                stop=False,
            )

            # ---- main loop over 256-row groups ----
            for g in range(n_groups):
                # load + cast fp32 -> bf16 during DMA (SWDGE)
                xb = xb_pool.tile([P, 2, F], bf16)
                x_src = x[g * 2 * P : (g + 1) * 2 * P, :].rearrange(
                    "(j p) f -> p j f", p=P
                )
                nc.gpsimd.dma_start(out=xb, in_=x_src)

                # transpose both 128-row subtiles, all 4 feature chunks
                pt = pt_pool.tile([P, 2, FC, P], bf16)
                for j in range(2):
                    for c in range(FC):
                        nc.tensor.matmul(
                            pt[:, j, c, :],
                            xb[:, j, c * P : (c + 1) * P],
                            ident,
                            is_transpose=True,
                            start=(j == 0 and c == 0),
                            stop=(j == 1 and c == FC - 1),
                        )
                # PSUM -> SBUF, relayout to [p, c, j*128+r]
                xt = xt_pool.tile([P, FC, 2 * P], bf16)
                for j in range(2):
                    nc.vector.tensor_copy(
                        out=xt[:, :, j * P : (j + 1) * P], in_=pt[:, j]
                    )

                # H^T[m*128+p, r] over 256 rows, accumulating feature chunks
                hps = []
                for m in range(MC):
                    hp = hp_pool.tile([P, 2 * P], f32, tag="hp")
                    hps.append(hp)
                    for c in range(FC):
                        nc.tensor.matmul(
                            hp,
                            w1b[:, c, m, :],
                            xt[:, c, :],
                            start=(c == 0),
                            stop=(c == FC - 1),
                        )

                # tanh(H^T + b1) -> bf16
                th = th_pool.tile([P, MC, 2, P], bf16)
                for m in range(MC):
                    for j in range(2):
                        nc.scalar.activation(
                            th[:, m, j, :],
                            hps[m][:, j * P : (j + 1) * P],
                            mybir.ActivationFunctionType.Tanh,
                            bias=b1s[:, m : m + 1],
                        )

                # s[r] = tanhH^T.T @ W2, then w = exp(s)
                wv = wv_pool.tile([P, 2], bf16)
                for j in range(2):
                    sp = sp_pool.tile([P, 1], f32, tag="sp")
                    for m in range(MC):
                        nc.tensor.matmul(
                            sp,
                            th[:, m, j, :],
                            w2b[:, m, :],
                            start=(m == 0),
                            stop=(m == MC - 1),
                        )
                    nc.scalar.activation(
                        wv[:, j : j + 1],
                        sp,
                        mybir.ActivationFunctionType.Exp,
                    )

                # per-bag accumulation
                for j in range(2):
                    t = 2 * g + j
                    col = tile_col[t]
                    nc.tensor.matmul(
                        z_psum[0:1, col : col + 1],
                        ones,
                        wv[:, j : j + 1],
                        start=False,
                        stop=False,
                    )
                    for c in range(FC):
                        nc.tensor.matmul(
                            ut_psum[:, c * n_cols + col : c * n_cols + col + 1],
                            xb[:, j, c * P : (c + 1) * P],
                            wv[:, j : j + 1],
                            start=False,
                            stop=False,
                        )

            # ---- epilogue: close accumulator groups, PSUM -> SBUF -> DRAM ----
            nc.tensor.matmul(
                ut_psum[:, 0 : FC * n_cols],
                ident,
                zeros[:, 0:1].broadcast_to([P, FC * n_cols]),
                start=False,
                stop=True,
            )
            nc.tensor.matmul(
                z_psum[0:1, 0:n_cols],
                ones,
                zeros[:, 0:1].broadcast_to([P, n_cols]),
                start=False,
                stop=True,
            )
            ut_sb = out_pool.tile([P, FC * n_cols], f32)
            nc.vector.tensor_copy(out=ut_sb, in_=ut_psum)
            z_sb = out_pool.tile([1, n_cols], f32)
            nc.vector.tensor_copy(out=z_sb, in_=z_psum)
            nc.sync.dma_start(out=ut_out[:, :], in_=ut_sb)
            nc.sync.dma_start(out=z_out[:, :], in_=z_sb)

    nc.compile()
    return nc


def x_ap_rearr(t, pattern, **axes):
    """rearrange a DRAM tensor handle's access pattern."""
    return t.rearrange(pattern, **axes)


def _run_device(X, W1, b1, W2, bag_rows, trace=False, trace_kwargs=None):
    from concourse.bass_utils import run_bass_kernel_spmd

    rows_per_core = X.shape[0] // N_CORES
    n_tiles = rows_per_core // P
    tiles_per_bag = bag_rows // P
    n_cols = n_tiles // tiles_per_bag
    tile_col = [t // tiles_per_bag for t in range(n_tiles)]

    key = (rows_per_core, bag_rows)
    if key in _COMPILED_CACHE:
        nc = _COMPILED_CACHE[key]
    else:
        nc = _build_program(n_tiles, tile_col, n_cols)
        _COMPILED_CACHE[key] = nc

    in_maps = []
    for c in range(N_CORES):
        in_maps.append(
            {
                "x": np.ascontiguousarray(
                    X[c * rows_per_core : (c + 1) * rows_per_core], np.float32
                ),
                "w1": np.ascontiguousarray(W1, np.float32),
                "b1": np.ascontiguousarray(b1, np.float32),
                "w2": np.ascontiguousarray(W2.reshape(HID, 1), np.float32),
            }
        )
    kw = dict(trace_kwargs or {})
    res = run_bass_kernel_spmd(
        nc, in_maps, list(range(N_CORES)), trace=trace, **kw
    )

    FC = F // P
    U = np.zeros((N_CORES * n_cols, F), np.float32)
    Z = np.float64(0.0)
    for c in range(N_CORES):
        ut = res.results[c]["ut"]  # [128, FC*n_cols]
        z = res.results[c]["z"]  # [1, n_cols]
        for cc in range(FC):
            # ut[p, cc*n_cols + b] = U^T[cc*128+p, b]
            U[c * n_cols : (c + 1) * n_cols, cc * P : (cc + 1) * P] = ut[
                :, cc * n_cols : (cc + 1) * n_cols
            ].T
        Z += np.float64(z.sum())
    return U, Z, res


def _kernel_numpy(instance_features, bag_sizes, W1, b1, W2, b2):
    """Exact-math fallback for bag layouts the device program doesn't cover."""
    X = np.asarray(instance_features, np.float32)
    s = np.tanh(X @ W1 + b1) @ W2.reshape(-1, 1) + np.asarray(b2).reshape(1, -1)
    s = s - s.max()
    w = np.exp(s)
    w = w / w.sum()
    offsets = np.cumsum(np.asarray(bag_sizes, np.int64))
    seg = np.searchsorted(offsets, np.arange(X.shape[0]), side="right")
    out = np.zeros((len(bag_sizes), X.shape[1]), np.float32)
    np.add.at(out, seg[seg < len(bag_sizes)], (X * w)[seg < len(bag_sizes)])
    return out


def kernel(**inputs):
    X = np.asarray(inputs["instance_features"], np.float32)
    bag_sizes = np.asarray(inputs["bag_sizes"], np.int64)
    W1 = np.asarray(inputs["W1"], np.float32)
    b1 = np.asarray(inputs["b1"], np.float32)
    W2 = np.asarray(inputs["W2"], np.float32)
    b2 = np.asarray(inputs["b2"], np.float32)

    T, Fdim = X.shape
    B = bag_sizes.shape[0]
    bag = int(bag_sizes[0]) if B else 0
    aligned = (
        Fdim == F
        and B > 0
        and np.all(bag_sizes == bag)
        and bag % P == 0
        and bag * B == T
        and T % N_CORES == 0
        and (T // N_CORES) % (2 * P) == 0
        and (T // N_CORES) % bag == 0
    )
    if not aligned:
        return _kernel_numpy(X, bag_sizes, W1, b1, W2, b2)

    U, Z, _ = _run_device(X, W1, b1, W2, bag)
    return (U / np.float32(Z)).astype(np.float32)


# revision 12
# speedup vs baseline: 1.1399x; 1.1399x over previous
"""AttentionMILPooling Trainium2 kernel.

Math (matches the jax reference):
    scores  = tanh(X @ W1 + b1) @ W2 + b2          # [T, 1]
    weights = softmax(scores, axis=0)              # global over all T
    out[b]  = sum_{i in bag b} weights[i] * X[i]   # [64, 512]

Key identities used:
  * b2 cancels exactly in the softmax (exp(s+b2)/sum exp(s+b2) = exp(s)/sum exp(s)).
  * scores are bounded (|s| <= sum|W2| ~ 13) so no max-subtraction is needed;
    exp stays well inside fp32 range.
  * out[b] = (sum_{i in b} exp(s_i) * X_i) / Z with Z = sum_i exp(s_i): each core
    computes unnormalized per-bag sums U and per-bag exp-sums z; the host sums z
    over all cores/bags and divides once.

Sharding: rows (instances) are split contiguously across 8 cores, 16384 rows
each; with the equal 2048-row bags every core owns exactly 8 whole bags.
The tiny MLP weights are replicated. Each core returns U^T [512 x 8] and
z [1 x 8]; the host concatenates, divides by the global Z.

Per-core device pipeline, per 256-row group (64 groups):
  DMA   : X rows fp32 -> SBUF bf16 (cast in SWDGE DMA)         [128,2,512]
  PE    : 8x transpose  X^T chunks -> PSUM (bf16)              [128,1024]
  DVE   : 2x copy PSUM->SBUF X^T                               [128,4,256]
  PE    : 8x matmul  H^T = W1^T @ X^T (accum over f chunks)    [128,512] f32
  ACT   : 4x tanh(H^T + b1) -> SBUF bf16
  PE    : 4x matmul  s = tanhH^T.T @ W2 (accum over h chunks)  [128,2] f32
  ACT   : 2x exp(s) -> w SBUF bf16
  PE    : 2x matmul  z[bag]  += ones.T @ w
  PE    : 8x matmul  U^T[:,bag] += X_chunk.T @ w  (per f chunk)
"""

import numpy as np

N_CORES = 8
T_FULL = 131072
F = 512  # feature dim
HID = 256  # hidden dim
B_FULL = 64  # number of bags
P = 128  # partitions

_COMPILED_CACHE = {}


def _build_program(n_tiles, tile_col, n_cols):
    """Build the SPMD bass program.

    n_tiles: number of 128-row tiles per core (must be divisible by 4).
    tile_col: list, local bag-column index for each tile (same on all cores).
    n_cols: number of local bag columns.
    """
    import concourse.bass as bass
    import concourse.bacc as bacc
    import concourse.mybir as mybir
    from concourse.tile import TileContext
    from concourse.masks import make_identity

    f32 = mybir.dt.float32
    bf16 = mybir.dt.bfloat16
    FC = F // P  # 4 feature chunks
    MC = HID // P  # 2 hidden chunks
    rows_per_core = n_tiles * P
    JT = 4  # 128-row subtiles per group
    GR = JT * P  # rows per group
    n_groups = n_tiles // JT

    nc = bacc.Bacc(
        "TRN2", target_bir_lowering=False, debug=False, num_devices=N_CORES
    )

    x = nc.declare_dram_parameter("x", [rows_per_core, F], f32, isOutput=False)
    w1 = nc.declare_dram_parameter("w1", [F, HID], f32, isOutput=False)
    b1 = nc.declare_dram_parameter("b1", [HID], f32, isOutput=False)
    w2 = nc.declare_dram_parameter("w2", [HID, 1], f32, isOutput=False)
    u_out = nc.declare_dram_parameter("u", [n_cols, F], f32, isOutput=True)
    w_out = nc.declare_dram_parameter("w", [P, n_tiles], f32, isOutput=True)

    with TileContext(nc) as tc:
        with (
            tc.tile_pool(name="const", bufs=1) as const_pool,
            tc.tile_pool(name="xb", bufs=3) as xb_pool,
            tc.tile_pool(name="xt", bufs=2) as xt_pool,
            tc.tile_pool(name="th", bufs=2) as th_pool,
            tc.tile_pool(name="out_sb", bufs=1) as out_pool,
            tc.tile_pool(name="pt", bufs=2, space="PSUM") as pt_pool,
            tc.tile_pool(name="hp", bufs=2, space="PSUM") as hp_pool,
            tc.tile_pool(name="sp", bufs=2, space="PSUM") as sp_pool,
            tc.tile_pool(name="acc", bufs=2, space="PSUM") as acc_pool,
        ):
            # ---- constants ----
            ident = const_pool.tile([P, P], bf16)
            make_identity(nc, ident)

            # W1 -> bf16 chunks: w1b[p, c, m, j] = W1[c*128+p, m*128+j]
            w1f = const_pool.tile([P, FC, MC, P], f32)
            nc.sync.dma_start(
                out=w1f, in_=w1.rearrange("(c p) (m j) -> p c m j", p=P, j=P)
            )
            w1b = const_pool.tile([P, FC, MC, P], bf16)
            nc.vector.tensor_copy(out=w1b, in_=w1f)

            # W2 -> bf16 chunks: w2b[p, m, 0] = W2[m*128+p, 0]
            w2f = const_pool.tile([P, MC, 1], f32)
            nc.sync.dma_start(
                out=w2f, in_=w2.rearrange("(m p) one -> p m one", p=P)
            )
            w2b = const_pool.tile([P, MC, 1], bf16)
            nc.vector.tensor_copy(out=w2b, in_=w2f)

            # b1 per-partition bias: b1s[p, m] = b1[m*128+p]
            b1s = const_pool.tile([P, MC], f32)
            nc.sync.dma_start(out=b1s, in_=b1.rearrange("(m p) -> p m", p=P))

            # softmax weights for every row, bf16 (also read back by the host
            # to form the global softmax denominator)
            wsave = const_pool.tile([P, n_tiles], bf16)

            # ---- per-bag accumulator handling ----
            # matmul outputs must start at partition 0/32/64, so each bag
            # accumulates in its own [1, F] PSUM tile (one bank) for the
            # contiguous run of its tiles, then is copied into its row of
            # the SBUF result before the tile is recycled.
            u_sb = out_pool.tile([1, n_cols, F], f32)
            first_tile = {}
            last_tile = {}
            for t, cl in enumerate(tile_col):
                first_tile.setdefault(cl, t)
                last_tile[cl] = t
            u_bag = [None] * n_cols

            # ---- main loop over 512-row groups ----
            for g in range(n_groups):
                # load + cast fp32 -> bf16 during DMA (SWDGE)
                xb = xb_pool.tile([P, JT, F], bf16)
                x_src = x[g * GR : (g + 1) * GR, :].rearrange(
                    "(j p) f -> p j f", p=P
                )
                nc.gpsimd.dma_start(out=xb, in_=x_src)

                # transpose all subtiles/chunks: two PSUM tiles of 8 each
                pts = []
                for h in range(2):
                    pt = pt_pool.tile([P, 2, FC, P], bf16, tag="pt")
                    pts.append(pt)
                    for jj in range(2):
                        j = 2 * h + jj
                        for c in range(FC):
                            nc.tensor.matmul(
                                pt[:, jj, c, :],
                                xb[:, j, c * P : (c + 1) * P],
                                ident,
                                is_transpose=True,
                                start=(jj == 0 and c == 0),
                                stop=(jj == 1 and c == FC - 1),
                            )
                # PSUM -> SBUF, relayout to [p, c, j*128+r]
                xt = xt_pool.tile([P, FC, JT * P], bf16)
                for h in range(2):
                    for jj in range(2):
                        j = 2 * h + jj
                        nc.vector.tensor_copy(
                            out=xt[:, :, j * P : (j + 1) * P], in_=pts[h][:, jj]
                        )

                # H^T[m*128+p, r] over 512 rows, accumulating feature chunks;
                # then tanh(H^T + b1) -> bf16 in one op per m-chunk
                th = th_pool.tile([P, MC, JT, P], bf16)
                for m in range(MC):
                    hp = hp_pool.tile([P, JT * P], f32, tag="hp")
                    for c in range(FC):
                        nc.tensor.matmul(
                            hp,
                            w1b[:, c, m, :],
                            xt[:, c, :],
                            start=(c == 0),
                            stop=(c == FC - 1),
                        )
                    nc.scalar.activation(
                        th[:, m],
                        hp.rearrange("p (j r) -> p j r", j=JT),
                        mybir.ActivationFunctionType.Tanh,
                        bias=b1s[:, m : m + 1],
                    )

                # s[r] = tanhH^T.T @ W2 for all 4 subtiles, one PSUM region
                sp = sp_pool.tile([P, JT], f32)
                for j in range(JT):
                    for m in range(MC):
                        nc.tensor.matmul(
                            sp[:, j : j + 1],
                            th[:, m, j, :],
                            w2b[:, m, :],
                            start=(j == 0 and m == 0),
                            stop=(j == JT - 1 and m == MC - 1),
                        )
                # w = exp(s) -> persistent wsave columns
                nc.scalar.activation(
                    wsave[:, g * JT : (g + 1) * JT],
                    sp,
                    mybir.ActivationFunctionType.Exp,
                )

                # per-bag accumulation: U[col, :] += w_tile^T @ X_tile
                for j in range(JT):
                    t = JT * g + j
                    col = tile_col[t]
                    if u_bag[col] is None:
                        u_bag[col] = acc_pool.tile([1, F], f32, name="u_bag", tag="u_bag")
                    nc.tensor.matmul(
                        u_bag[col],
                        wsave[:, t : t + 1],
                        xb[:, j, :],
                        start=(t == first_tile[col]),
                        stop=(t == last_tile[col]),
                    )
                    if t == last_tile[col]:
                        nc.vector.tensor_copy(
                            out=u_sb[:, col, :], in_=u_bag[col]
                        )
                        u_bag[col] = None

            # ---- epilogue: DMA results out ----
            nc.sync.dma_start(
                out=u_out.rearrange("(o b) f -> o b f", o=1), in_=u_sb
            )
            wf = out_pool.tile([P, n_tiles], f32)
            nc.vector.tensor_copy(out=wf, in_=wsave)
            nc.sync.dma_start(out=w_out[:, :], in_=wf)

    nc.compile()
    return nc


def x_ap_rearr(t, pattern, **axes):
    """rearrange a DRAM tensor handle's access pattern."""
    return t.rearrange(pattern, **axes)


def _run_device(X, W1, b1, W2, bag_rows, trace=False, trace_kwargs=None):
    from concourse.bass_utils import run_bass_kernel_spmd

    rows_per_core = X.shape[0] // N_CORES
    n_tiles = rows_per_core // P
    tiles_per_bag = bag_rows // P
    n_cols = n_tiles // tiles_per_bag
    tile_col = [t // tiles_per_bag for t in range(n_tiles)]

    key = (rows_per_core, bag_rows)
    if key in _COMPILED_CACHE:
        nc = _COMPILED_CACHE[key]
    else:
        nc = _build_program(n_tiles, tile_col, n_cols)
        _COMPILED_CACHE[key] = nc

    in_maps = []
    for c in range(N_CORES):
        in_maps.append(
            {
                "x": np.ascontiguousarray(
                    X[c * rows_per_core : (c + 1) * rows_per_core], np.float32
                ),
                "w1": np.ascontiguousarray(W1, np.float32),
                "b1": np.ascontiguousarray(b1, np.float32),
                "w2": np.ascontiguousarray(W2.reshape(HID, 1), np.float32),
            }
        )
    kw = dict(trace_kwargs or {})
    res = run_bass_kernel_spmd(
        nc, in_maps, list(range(N_CORES)), trace=trace, **kw
    )

    U = np.zeros((N_CORES * n_cols, F), np.float32)
    Z = np.float64(0.0)
    for c in range(N_CORES):
        U[c * n_cols : (c + 1) * n_cols] = res.results[c]["u"]
        Z += np.float64(res.results[c]["w"]).sum()
    return U, Z, res


def _kernel_numpy(instance_features, bag_sizes, W1, b1, W2, b2):
    """Exact-math fallback for bag layouts the device program doesn't cover."""
    X = np.asarray(instance_features, np.float32)
    s = np.tanh(X @ W1 + b1) @ W2.reshape(-1, 1) + np.asarray(b2).reshape(1, -1)
    s = s - s.max()
    w = np.exp(s)
    w = w / w.sum()
    offsets = np.cumsum(np.asarray(bag_sizes, np.int64))
    seg = np.searchsorted(offsets, np.arange(X.shape[0]), side="right")
    out = np.zeros((len(bag_sizes), X.shape[1]), np.float32)
    np.add.at(out, seg[seg < len(bag_sizes)], (X * w)[seg < len(bag_sizes)])
    return out


def kernel(**inputs):
    X = np.asarray(inputs["instance_features"], np.float32)
    bag_sizes = np.asarray(inputs["bag_sizes"], np.int64)
    W1 = np.asarray(inputs["W1"], np.float32)
    b1 = np.asarray(inputs["b1"], np.float32)
    W2 = np.asarray(inputs["W2"], np.float32)
    b2 = np.asarray(inputs["b2"], np.float32)

    T, Fdim = X.shape
    B = bag_sizes.shape[0]
    bag = int(bag_sizes[0]) if B else 0
    aligned = (
        Fdim == F
        and B > 0
        and np.all(bag_sizes == bag)
        and bag % P == 0
        and bag * B == T
        and T % N_CORES == 0
        and (T // N_CORES) % (2 * P) == 0
        and (T // N_CORES) % bag == 0
    )
    if not aligned:
        return _kernel_numpy(X, bag_sizes, W1, b1, W2, b2)

    U, Z, _ = _run_device(X, W1, b1, W2, bag)
    return (U / np.float32(Z)).astype(np.float32)
